# revision 1
# baseline (speedup 1.0000x reference)
"""TRN2 Bass kernel for nn_Conv2d_62826781606523 (LUT-conv / gnn message passing).

Math: for each table t=(co,p,f) with K_LUT=2 inputs (a,b) and weights w[t,0:4]:
    out_t = sum_j w_j (1+a*s0j)(1+b*s1j)  with  s0=(-,-,+,+), s1=(-,+,-,+)
          = c0 + c1*a + c2*b + c3*a*b
    c0 =  w0+w1+w2+w3, c1 = -w0-w1+w2+w3, c2 = -w0+w1-w2+w3, c3 = w0-w1-w2+w3
    out[b,co,p] = sum_f out_t
`a` is the regular im2col element E[b,p,f]; `b` is E[b,p,sel2[co,p,f]] where
sel2 is a static within-receptive-field index derived from `mask`.

Sharding: tensor-parallel over output channels, 4 of 32 per core (8 cores).
Host does index/layout marshalling (im2col + static-index gather + bf16 pack);
the device streams weights + operands and does all arithmetic:
  butterfly (c1,c2,c3,bias), products, 144-wide segment reductions.
"""
import numpy as np
import ml_dtypes

import concourse.bass as bass
import concourse.bacc as bacc
import concourse.mybir as mybir
from concourse.bass_types import AP
from concourse.tile import TileContext
from concourse.bass_utils import run_bass_kernel_spmd

# problem constants (hardcoded per task contract)
B, CIN, COUT, KS, H, W = 4, 16, 32, 3, 32, 32
HOUT = WOUT = 30
P = HOUT * WOUT          # 900
F = CIN * KS * KS        # 144
T = COUT * P * F
NCORE = 8
CO_BLK = COUT // NCORE   # 4
PPAD = 1024              # p padded to 8 tiles of 128
NT = PPAD // 128         # 8 p-tiles
CF = CO_BLK * F          # 576
BCF = B * CF             # 2304
BF16 = mybir.dt.bfloat16
F32 = mybir.dt.float32

_cache = {}


def _bcast(ap, n, pos):
    """Insert a 0-stride dim of size n at free-dim position pos (1-based
    within ap.ap list after the partition dim)."""
    new = list(ap.ap)
    new.insert(pos, [0, n])
    return AP(ap.tensor, ap.offset, new)


def _build():
    nc = bacc.Bacc()
    d_w = nc.dram_tensor("w", [PPAD, 4 * CF], BF16, kind="ExternalInput")
    d_av = nc.dram_tensor("av", [PPAD, B * F], BF16, kind="ExternalInput")
    d_bv = nc.dram_tensor("bv", [PPAD, BCF], BF16, kind="ExternalInput")
    d_out = nc.dram_tensor("out", [PPAD, B * CO_BLK], F32, kind="ExternalOutput")

    mul = mybir.AluOpType.mult
    add = mybir.AluOpType.add

    with TileContext(nc) as tc:
        with (
            tc.tile_pool(name="io", bufs=3) as io,
            tc.tile_pool(name="wk", bufs=2) as wk,
        ):
            for i in range(NT):
                pr = bass.ts(i, 128)
                wt = io.tile([128, 4 * CF], BF16, tag="wt")
                at = io.tile([128, B * F], BF16, tag="at")
                bt = io.tile([128, BCF], BF16, tag="bt")
                nc.sync.dma_start(wt[:], d_w[pr, :])
                nc.sync.dma_start(at[:], d_av[pr, :])
                nc.sync.dma_start(bt[:], d_bv[pr, :])

                w_ = [wt[:, bass.ts(j, CF)] for j in range(4)]
                tA = wk.tile([128, CF], BF16, tag="tA")
                tB = wk.tile([128, CF], BF16, tag="tB")
                tC = wk.tile([128, CF], BF16, tag="tC")
                tD = wk.tile([128, CF], BF16, tag="tD")
                c1 = wk.tile([128, CF], BF16, tag="c1")
                c2 = wk.tile([128, CF], BF16, tag="c2")
                c3 = wk.tile([128, CF], BF16, tag="c3")
                nc.vector.tensor_add(tA[:], w_[0], w_[1])
                nc.vector.tensor_add(tB[:], w_[2], w_[3])
                nc.vector.tensor_sub(tC[:], w_[1], w_[0])
                nc.vector.tensor_sub(tD[:], w_[3], w_[2])
                nc.vector.tensor_sub(c1[:], tB[:], tA[:])
                nc.vector.tensor_add(c2[:], tC[:], tD[:])
                nc.vector.tensor_sub(c3[:], tD[:], tC[:])
                # bias[co] = sum_f c0, c0 = A+B
                t0 = wk.tile([128, CF], BF16, tag="t0")
                bias = wk.tile([128, CO_BLK], F32, tag="bias")
                nc.vector.tensor_add(t0[:], tA[:], tB[:])
                nc.vector.tensor_reduce(
                    bias[:], t0[:].rearrange("p (c f) -> p c f", f=F),
                    mybir.AxisListType.X, add,
                )

                # replicate a across co: ar[(b,co,f)] = a[(b,f)]
                ar = wk.tile([128, BCF], BF16, tag="ar")
                ar4 = ar[:].rearrange("p (b c f) -> p b c f", b=B, c=CO_BLK)
                at3 = at[:].rearrange("p (b f) -> p b f", b=B)
                for co in range(CO_BLK):
                    nc.vector.tensor_copy(ar4[:, :, co, :], at3)

                # out_t = a*(c1 + c3*b) + c2*b  — per-b flat contiguous APs so
                # the bf16 2x DVE mode engages; one fewer V-pass than the
                # naive c1*a + c2*b + c3*a*b form.
                q = wk.tile([128, BCF], BF16, tag="q")
                s2 = wk.tile([128, BCF], BF16, tag="s2")
                t1 = wk.tile([128, BCF], BF16, tag="t1")
                for b in range(B):
                    bs = bass.ts(b, CF)
                    nc.vector.tensor_tensor(q[:, bs], c3[:], bt[:, bs], mul)
                    nc.vector.tensor_add(q[:, bs], q[:, bs], c1[:])
                    nc.vector.tensor_tensor(s2[:, bs], c2[:], bt[:, bs], mul)
                nc.vector.tensor_tensor(t1[:], q[:], ar[:], mul)
                nc.vector.tensor_add(t1[:], t1[:], s2[:])

                # segment reduce over f and add bias (broadcast over b)
                red = wk.tile([128, B * CO_BLK], F32, tag="red")
                nc.vector.tensor_reduce(
                    red[:], t1[:].rearrange("p (q f) -> p q f", f=F),
                    mybir.AxisListType.X, add,
                )
                ot = io.tile([128, B * CO_BLK], F32, tag="ot")
                nc.vector.tensor_add(
                    ot[:].rearrange("p (b c) -> p b c", b=B),
                    red[:].rearrange("p (b c) -> p b c", b=B),
                    _bcast(bias[:], B, 1),
                )
                nc.sync.dma_start(d_out[pr, :], ot[:])
    nc.finalize()
    return nc


def _prep(x, weight, mask):
    x = np.ascontiguousarray(np.asarray(x, dtype=np.float32))
    weight = np.ascontiguousarray(np.asarray(weight, dtype=np.float32))
    mask = np.asarray(mask, dtype=np.int64)

    # within-receptive-field index of LUT input 2 (input 1 is the regular
    # im2col element f — asserted below)
    m = mask.reshape(COUT, P, F, 2, 3)
    pr = (np.arange(P) // WOUT)[None, :, None]
    pc = (np.arange(P) % WOUT)[None, :, None]
    g = (m[..., 0] * KS + (m[..., 1] - pr[..., None])) * KS + (m[..., 2] - pc[..., None])
    sel2 = g[..., 1].astype(np.int64)               # (COUT,P,F)

    # im2col E[b,p,f]
    E = np.empty((B, P, F), dtype=np.float32)
    xv = x.reshape(B, CIN, H, W)
    for gg in range(F):
        cch, rem = divmod(gg, KS * KS)
        ddr, ddc = divmod(rem, KS)
        E[:, :, gg] = xv[:, cch, ddr:ddr + HOUT, ddc:ddc + WOUT].reshape(B, P)

    # gather of input-2 values: bvals[b,co,p,f] = E[b,p,sel2[co,p,f]]
    flat_idx = (np.arange(P)[None, :, None] * F + sel2).reshape(-1)  # (COUT*P*F)
    bvals = E.reshape(B, P * F)[:, flat_idx].reshape(B, COUT, P, F)

    bf = ml_dtypes.bfloat16
    # av: (PPAD, B, F)
    av = np.zeros((PPAD, B, F), dtype=bf)
    av[:P] = E.transpose(1, 0, 2)
    av = av.reshape(PPAD, B * F)

    w4 = weight.reshape(COUT, P, F, 4)
    in_maps = []
    for mcore in range(NCORE):
        cos = slice(mcore * CO_BLK, (mcore + 1) * CO_BLK)
        wp = np.zeros((PPAD, 4, CO_BLK, F), dtype=bf)
        wp[:P] = w4[cos].transpose(1, 3, 0, 2)       # (P, j, co, f)
        bv = np.zeros((PPAD, B, CO_BLK, F), dtype=bf)
        bv[:P] = bvals[:, cos].transpose(2, 0, 1, 3)  # (P, b, co, f)
        in_maps.append({
            "w": wp.reshape(PPAD, 4 * CF),
            "av": av,
            "bv": bv.reshape(PPAD, BCF),
        })
    return in_maps


def kernel(x, weight, mask):
    if "nc" not in _cache:
        _cache["nc"] = _build()
    nc = _cache["nc"]
    in_maps = _prep(x, weight, mask)
    res = run_bass_kernel_spmd(nc, in_maps, core_ids=list(range(NCORE)))
    out = np.empty((B, COUT, HOUT, WOUT), dtype=np.float32)
    for mcore in range(NCORE):
        dev = res.results[mcore]["out"][:P]          # (900, B*CO_BLK)
        dev = dev.reshape(P, B, CO_BLK).transpose(1, 2, 0)
        out[:, mcore * CO_BLK:(mcore + 1) * CO_BLK] = dev.reshape(
            B, CO_BLK, HOUT, WOUT)
    return out


if __name__ == "__main__":
    rng = np.random.default_rng(0)
    x = rng.standard_normal((B, CIN, H, W), dtype=np.float32)
    weight = rng.standard_normal((T, 4), dtype=np.float32)
    # quick self-test with a synthetic valid mask is not meaningful; use test.py
    print("kernel module ok")



# revision 5
# speedup vs baseline: 1.8550x; 1.8550x over previous
"""TRN2 Bass kernel for nn_Conv2d_62826781606523 (LUT-conv, K_LUT=2).

Math per table t=(co,p,f), weights w[t,0:4], inputs a=E[b,p,f], bb=E[b,p,sel2]:
    out_t = c0 + c1*a + c2*bb + c3*a*bb         (butterfly of w)
    out[b,co,p] = sum_f out_t
Host-side weight-only preprocessing (offline-foldable):
    bias[co,p] = sum_f c0;   c12 = c1 + scatter_add(c2 over sel2)
so   out[b,co,p] = bias + sum_f c12[co,p,f]*a[b,p,f] + sum_f c3*(a*bb).

Device split (p-sharded across 8 cores, PPC=116 positions/core):
  DVE   : m = bv*ar, t = m*c3  (bf16 2x-mode flat/broadcast instrs)
  PE    : c12-term as per-p stationary matmuls over f (K=128+17 incl bias row)
          + segmented f-reduction of t via one-hot stationaries; everything
          accumulates into one PSUM tile (32co x 464(b,p)).
  Output: single PSUM -> SBUF copy -> DMA.
Layout: partitions = (co_local8, f16) per co-octet; f = 144 -> 9 "ninths".
Host does layout only for x-dependent data: im2col, static-index gather,
bf16 pack, padding. No x-dependent arithmetic on host.
"""
import numpy as np
import ml_dtypes

import concourse.bass as bass
import concourse.bacc as bacc
import concourse.mybir as mybir
from concourse.bass_types import AP
from concourse.tile import TileContext
from concourse.bass_utils import run_bass_kernel_spmd

# problem constants (hardcoded per task contract)
B, CIN, COUT, KS, H, W = 4, 16, 32, 3, 32, 32
HOUT = WOUT = 30
P = HOUT * WOUT          # 900
F = CIN * KS * KS        # 144
T = COUT * P * F
NCORE = 8
PPC = 116                # p positions per core (padded; 4x29 quads, 4*116=464<=512)
NB = PPC * B             # 464 moving columns (b-major: col = b*PPC + p)
NOCT = 4                 # co octets of 8
NINTH = 9                # f = 9 * 16
LANES = 128              # (co_local 8) x (f16)
FA = 128                 # f-chunk A rows for c12 matmul
FB = F - FA + 1          # f-chunk B rows + 1 bias row = 17
BF16 = mybir.dt.bfloat16
F32 = mybir.dt.float32

# per-core valid p counts (sum = 900)
PCNT = [113, 113, 113, 113, 112, 112, 112, 112]
PSTART = np.concatenate([[0], np.cumsum(PCNT)[:-1]]).astype(int)

_cache = {}


def _bcast(ap, n, pos):
    """Insert a 0-stride dim of size n at free-dim position pos."""
    new = list(ap.ap)
    new.insert(pos, [0, n])
    return AP(ap.tensor, ap.offset, new)


def _build():
    nc = bacc.Bacc()
    d_w3 = [nc.dram_tensor(f"w3_{o}", [LANES, NINTH * PPC], BF16, kind="ExternalInput")
            for o in range(NOCT)]
    d_bv = [nc.dram_tensor(f"bv_{o}", [LANES, NINTH * NB], BF16, kind="ExternalInput")
            for o in range(NOCT)]
    d_ar = nc.dram_tensor("ar", [LANES, NINTH * NB], BF16, kind="ExternalInput")
    d_c12a = nc.dram_tensor("c12a", [FA, PPC * 32], BF16, kind="ExternalInput")
    d_c12b = nc.dram_tensor("c12b", [FB, PPC * 32], BF16, kind="ExternalInput")
    d_avpa = nc.dram_tensor("avpa", [FA, NB], BF16, kind="ExternalInput")
    d_avpb = nc.dram_tensor("avpb", [FB, NB], BF16, kind="ExternalInput")
    d_s = nc.dram_tensor("sred", [LANES, NOCT * 32], BF16, kind="ExternalInput")
    d_out = nc.dram_tensor("out", [32, NB], F32, kind="ExternalOutput")

    mul = mybir.AluOpType.mult

    with TileContext(nc) as tc:
        with (
            tc.tile_pool(name="cst", bufs=1) as cst,
            tc.tile_pool(name="io", bufs=2) as io,
            tc.tile_pool(name="wk", bufs=2) as wk,
            tc.psum_pool(name="ps", bufs=1) as ps,
        ):
            # --- resident inputs ---
            ar = cst.tile([LANES, NINTH * NB], BF16, name="art")
            avpa = cst.tile([FA, NB], BF16, name="avpat")
            avpb = cst.tile([FB, NB], BF16, name="avpbt")
            c12a = cst.tile([FA, PPC * 32], BF16, name="c12at")
            c12b = cst.tile([FB, PPC * 32], BF16, name="c12bt")
            sred = cst.tile([LANES, NOCT * 32], BF16, name="sredt")
            psum = ps.tile([32, NB], F32, name="psumt")
            out_sb = cst.tile([32, NB], F32, name="outsb")

            # first octet's data + m-pass operands first, then PE-phase inputs
            bv0 = io.tile([LANES, NINTH * NB], BF16, tag="bv")
            w30 = io.tile([LANES, NINTH * PPC], BF16, tag="w3")
            CH = 3  # DMA/compute chunks of 3 ninths
            CNB = CH * NB
            CNP = CH * PPC
            for c in range(NINTH // CH):
                nc.sync.dma_start(bv0[:, bass.ts(c, CNB)], d_bv[0][:, bass.ts(c, CNB)])
                nc.sync.dma_start(ar[:, bass.ts(c, CNB)], d_ar[:, bass.ts(c, CNB)])
            nc.sync.dma_start(w30[:], d_w3[0][:])
            nc.sync.dma_start(avpa[:], d_avpa[:])
            nc.sync.dma_start(avpb[:], d_avpb[:])
            nc.sync.dma_start(c12a[:], d_c12a[:])
            nc.sync.dma_start(c12b[:], d_c12b[:])
            nc.sync.dma_start(sred[:], d_s[:])

            # --- PE phase 1: c12 term (+bias via ones-row), accumulate into psum ---
            psum3 = psum[:].rearrange("q (b p) -> q b p", p=PPC)
            avpa3 = avpa[:].rearrange("f (b p) -> f b p", p=PPC)
            avpb3 = avpb[:].rearrange("f (b p) -> f b p", p=PPC)
            for p in range(PPC):
                # psum[co, b*PPC+p] += sum_f c12[f,(p,co)] * avp[f,(b,p)]
                # start=True ONLY on the very first matmul: it marks the whole
                # PSUM zero-region pending-zero; everything else accumulates.
                nc.tensor.matmul(psum3[:, :, p], c12a[:, bass.ts(p, 32)],
                                 avpa3[:, :, p],
                                 start=(p == 0), stop=False, skip_group_check=True)
                nc.tensor.matmul(psum3[:, :, p], c12b[:, bass.ts(p, 32)],
                                 avpb3[:, :, p],
                                 start=False, stop=False, skip_group_check=True)

            # --- per-octet pipeline ---
            bv_t, w3_t = bv0, w30
            for o in range(NOCT):
                if o + 1 < NOCT:
                    bv_n = io.tile([LANES, NINTH * NB], BF16, tag="bv")
                    w3_n = io.tile([LANES, NINTH * PPC], BF16, tag="w3")
                    for c in range(NINTH // CH):
                        nc.sync.dma_start(bv_n[:, bass.ts(c, CNB)],
                                          d_bv[o + 1][:, bass.ts(c, CNB)])
                    nc.sync.dma_start(w3_n[:], d_w3[o + 1][:])
                else:
                    bv_n = w3_n = None

                m_t = wk.tile([LANES, NINTH * NB], BF16, tag="m")
                t_t = wk.tile([LANES, NINTH * NB], BF16, tag="t")
                m4 = m_t[:].rearrange("l (n b p) -> l n b p", b=B, p=PPC)
                t4 = t_t[:].rearrange("l (n b p) -> l n b p", b=B, p=PPC)
                w3r = w3_t[:].rearrange("l (n p) -> l n p", p=PPC)
                for c in range(NINTH // CH):
                    cs = bass.ts(c, CNB)
                    csl = slice(c * CH, (c + 1) * CH)
                    nc.vector.tensor_tensor(m_t[:, cs], bv_t[:, cs], ar[:, cs], mul)
                    # w3 broadcast over b: dims (n, b(0-stride), p)
                    w3b = w3r[:, csl].unsqueeze(2).broadcast_to(
                        [LANES, CH, B, PPC])
                    nc.vector.tensor_tensor(t4[:, csl], m4[:, csl], w3b, mul)

                for n in range(NINTH):
                    last = (o == NOCT - 1) and (n == NINTH - 1)
                    nc.tensor.matmul(
                        psum[:], sred[:, bass.ts(o, 32)], t_t[:, bass.ts(n, NB)],
                        start=False, stop=last, skip_group_check=True)
                bv_t, w3_t = bv_n, w3_n

            # --- output: PSUM -> SBUF -> DRAM ---
            nc.scalar.copy(out_sb[:], psum[:])
            nc.sync.dma_start(d_out[:, :], out_sb[:])
    nc.finalize()
    return nc


def _prep(x, weight, mask):
    x = np.ascontiguousarray(np.asarray(x, dtype=np.float32))
    weight = np.ascontiguousarray(np.asarray(weight, dtype=np.float32))
    mask = np.asarray(mask, dtype=np.int64)
    bf = ml_dtypes.bfloat16

    # within-receptive-field index of LUT input 2
    m = mask.reshape(COUT, P, F, 2, 3)
    pr = (np.arange(P) // WOUT)[None, :, None]
    pc = (np.arange(P) % WOUT)[None, :, None]
    g = (m[..., 0] * KS + (m[..., 1] - pr[..., None])) * KS + (m[..., 2] - pc[..., None])
    sel2 = g[..., 1].astype(np.int64)                # (COUT,P,F)

    # im2col E[b,p,f]
    E = np.empty((B, P, F), dtype=np.float32)
    xv = x.reshape(B, CIN, H, W)
    for gg in range(F):
        cch, rem = divmod(gg, KS * KS)
        ddr, ddc = divmod(rem, KS)
        E[:, :, gg] = xv[:, cch, ddr:ddr + HOUT, ddc:ddc + WOUT].reshape(B, P)

    # gather of input-2 values: bvals[b,co,p,f] = E[b,p,sel2[co,p,f]]
    flat_idx = (np.arange(P)[None, :, None] * F + sel2).reshape(-1)
    bvals = E.reshape(B, P * F)[:, flat_idx].reshape(B, COUT, P, F)

    # weight-only preprocessing: butterfly + scatter + bias (offline-foldable)
    w4 = weight.reshape(COUT, P, F, 4)
    w0, w1, w2, w3_ = w4[..., 0], w4[..., 1], w4[..., 2], w4[..., 3]
    c0 = w0 + w1 + w2 + w3_
    c1 = -w0 - w1 + w2 + w3_
    c2 = -w0 + w1 - w2 + w3_
    c3 = w0 - w1 - w2 + w3_
    bias = c0.sum(-1)                                # (COUT,P)
    c12 = c1.copy()
    base = (np.arange(COUT * P) * F)[:, None]
    np.add.at(c12.reshape(-1), (base + sel2.reshape(COUT * P, F)).ravel(),
              c2.reshape(COUT * P, F).ravel())

    # one-hot reduction stationaries: S[lane, o*32+cog] = (8*o + lane//16 == cog)
    lane_co = (np.arange(LANES) // 16)
    sred = np.zeros((LANES, NOCT * 32), dtype=bf)
    for o in range(NOCT):
        sred[np.arange(LANES), o * 32 + 8 * o + lane_co] = 1.0

    in_maps = []
    for k in range(NCORE):
        ps_, ncnt = PSTART[k], PCNT[k]
        sl = slice(ps_, ps_ + ncnt)
        # padded p-local arrays
        Ek = np.zeros((B, PPC, F), dtype=np.float32)
        Ek[:, :ncnt] = E[:, sl]
        bvk = np.zeros((B, COUT, PPC, F), dtype=np.float32)
        bvk[:, :, :ncnt] = bvals[:, :, sl]
        c3k = np.zeros((COUT, PPC, F), dtype=np.float32)
        c3k[:, :ncnt] = c3[:, sl]
        c12k = np.zeros((COUT, PPC, F), dtype=np.float32)
        c12k[:, :ncnt] = c12[:, sl]
        biask = np.zeros((COUT, PPC), dtype=np.float32)
        biask[:, :ncnt] = bias[:, sl]

        im = {"sred": sred}
        # lanes = (co_local, j16); f = n*16 + j
        E_l = Ek.reshape(B, PPC, NINTH, 16)          # (b,p,n,j)
        art = np.ascontiguousarray(
            np.broadcast_to(E_l.transpose(3, 2, 0, 1)[None], (8, 16, NINTH, B, PPC))
        ).reshape(LANES, NINTH * NB).astype(bf)
        im["ar"] = art
        for o in range(NOCT):
            cosl = slice(8 * o, 8 * o + 8)
            bvo = bvk[:, cosl].reshape(B, 8, PPC, NINTH, 16)
            im[f"bv_{o}"] = np.ascontiguousarray(
                bvo.transpose(1, 4, 3, 0, 2)).reshape(LANES, NINTH * NB).astype(bf)
            w3o = c3k[cosl].reshape(8, PPC, NINTH, 16)
            im[f"w3_{o}"] = np.ascontiguousarray(
                w3o.transpose(0, 3, 2, 1)).reshape(LANES, NINTH * PPC).astype(bf)
        # c12 stationaries: [f, (p,co)]
        c12f = np.ascontiguousarray(c12k.transpose(2, 1, 0))     # (F, PPC, COUT)
        im["c12a"] = c12f[:FA].reshape(FA, PPC * 32).astype(bf)
        c12bk = np.empty((FB, PPC, 32), dtype=np.float32)
        c12bk[:FB - 1] = c12f[FA:]
        c12bk[FB - 1] = biask.T                                  # bias row
        im["c12b"] = c12bk.reshape(FB, PPC * 32).astype(bf)
        # avp moving: [f, (b,p)] b-major cols
        avf = np.ascontiguousarray(Ek.transpose(2, 0, 1))        # (F, B, PPC)
        im["avpa"] = avf[:FA].reshape(FA, NB).astype(bf)
        avpbk = np.empty((FB, B, PPC), dtype=np.float32)
        avpbk[:FB - 1] = avf[FA:]
        avpbk[FB - 1] = 1.0                                      # bias ones-row
        im["avpb"] = avpbk.reshape(FB, NB).astype(bf)
        in_maps.append(im)
    return in_maps


def kernel(x, weight, mask):
    if "nc" not in _cache:
        _cache["nc"] = _build()
    nc = _cache["nc"]
    in_maps = _prep(x, weight, mask)
    res = run_bass_kernel_spmd(nc, in_maps, core_ids=list(range(NCORE)))
    out = np.empty((B, COUT, HOUT, WOUT), dtype=np.float32)
    ov = out.reshape(B, COUT, P)
    for k in range(NCORE):
        dev = res.results[k]["out"].reshape(32, B, PPC)          # (co, b, p)
        ov[:, :, PSTART[k]:PSTART[k] + PCNT[k]] = dev.transpose(1, 0, 2)[:, :, :PCNT[k]]
    return out


if __name__ == "__main__":
    print("kernel module ok")


# revision 8
# speedup vs baseline: 2.2097x; 1.1912x over previous
"""TRN2 Bass kernel for nn_Conv2d_62826781606523 (LUT-conv, K_LUT=2).

Math per table t=(co,p,f), weights w[t,0:4], inputs a=E[b,p,f], bb=E[b,p,sel2]:
    out_t = c0 + c1*a + c2*bb + c3*a*bb         (butterfly of w)
    out[b,co,p] = sum_f out_t
Host-side weight-only preprocessing (offline-foldable):
    bias[co,p] = sum_f c0;   c12 = c1 + scatter_add(c2 over sel2)
so   out[b,co,p] = bias + sum_f (c3*bb + c12) * a.

Device split (p-sharded across 8 cores, PPC=116 positions/core):
  DVE   : q = c3*bv (b-bcast), q += c12 (b-bcast), t = q*ar   [bf16 2x mode]
  GpSimd: part of the q+=c12 pass + final bias add (psum f32)
  PE    : segmented f-reduction of t via one-hot stationaries, accumulating
          all 4 co-octets x 9 f-ninths into one PSUM tile (32co x 464(b,p))
  Output: PSUM + bias -> SBUF (GpSimd) -> DMA.
Layout: partitions = (co_local8, f16) per co-octet; f = 144 -> 9 "ninths".
Host does layout only for x-dependent data (im2col, static-index gather,
bf16 pack, padding); no x-dependent arithmetic on host.
"""
import numpy as np
import ml_dtypes

import concourse.bass as bass
import concourse.bacc as bacc
import concourse.mybir as mybir
from concourse.bass_types import AP
from concourse.tile import TileContext
from concourse.bass_utils import run_bass_kernel_spmd

# problem constants (hardcoded per task contract)
B, CIN, COUT, KS, H, W = 4, 16, 32, 3, 32, 32
HOUT = WOUT = 30
P = HOUT * WOUT          # 900
F = CIN * KS * KS        # 144
T = COUT * P * F
NCORE = 8
PPC = 116                # padded p positions per core
NB = PPC * B             # 464 columns (b-major: col = b*PPC + p), <=512 moving
NOCT = 4                 # co octets of 8
NINTH = 9                # f = 9 * 16
LANES = 128              # (co_local 8) x (f16)
BF16 = mybir.dt.bfloat16
F32 = mybir.dt.float32

PCNT = [113, 113, 113, 113, 112, 112, 112, 112]
PSTART = np.concatenate([[0], np.cumsum(PCNT)[:-1]]).astype(int)

CH = 3                   # ninths per DVE chunk
NCHUNK = NINTH // CH
CNB = CH * NB
# which (octet, chunk) of the add-pass runs on GpSimd instead of DVE
POOL_ADD = {(0, 2), (1, 2), (2, 2), (3, 2)}

_cache = {}


def _build():
    nc = bacc.Bacc()
    d_w3 = [nc.dram_tensor(f"w3_{o}", [LANES, NINTH * PPC], BF16, kind="ExternalInput")
            for o in range(NOCT)]
    d_c12 = [nc.dram_tensor(f"c12_{o}", [LANES, NINTH * PPC], BF16,
                            kind="ExternalInput")
             for o in range(NOCT)]
    d_bv = [nc.dram_tensor(f"bv_{o}", [LANES, NINTH * NB], BF16, kind="ExternalInput")
            for o in range(NOCT)]
    d_ar = nc.dram_tensor("ar", [LANES, NINTH * NB], BF16, kind="ExternalInput")
    d_bias = nc.dram_tensor("biasv", [32, PPC], F32, kind="ExternalInput")
    d_s = nc.dram_tensor("sred", [LANES, NOCT * 32], BF16, kind="ExternalInput")
    d_out = nc.dram_tensor("out", [32, NB], F32, kind="ExternalOutput")

    mul = mybir.AluOpType.mult
    add = mybir.AluOpType.add

    with TileContext(nc) as tc:
        with (
            tc.tile_pool(name="cst", bufs=1) as cst,
            tc.tile_pool(name="io", bufs=2) as io,
            tc.tile_pool(name="wk", bufs=2) as wk,
            tc.psum_pool(name="ps", bufs=1) as ps,
        ):
            # resident tiles
            ar = cst.tile([LANES, NINTH * NB], BF16, name="art")
            biast = cst.tile([32, PPC], F32, name="biastt")
            sred = cst.tile([LANES, NOCT * 32], BF16, name="sredt")
            psum = ps.tile([32, NB], F32, name="psumt")
            out_sb = cst.tile([32, NB], F32, name="outsb")

            # first octet + shared streams
            bv0 = io.tile([LANES, NINTH * NB], BF16, tag="bv")
            w30 = io.tile([LANES, NINTH * PPC], BF16, tag="w3")
            c120 = io.tile([LANES, NINTH * PPC], BF16, tag="c12")
            for c in range(NCHUNK):
                nc.sync.dma_start(bv0[:, bass.ts(c, CNB)], d_bv[0][:, bass.ts(c, CNB)])
                nc.sync.dma_start(ar[:, bass.ts(c, CNB)], d_ar[:, bass.ts(c, CNB)])
            nc.sync.dma_start(w30[:], d_w3[0][:])
            nc.sync.dma_start(c120[:], d_c12[0][:])
            nc.sync.dma_start(sred[:], d_s[:])
            nc.sync.dma_start(biast[:], d_bias[:])

            first_mm = True
            bv_t, w3_t, c12_t = bv0, w30, c120
            for o in range(NOCT):
                if o + 1 < NOCT:
                    bv_n = io.tile([LANES, NINTH * NB], BF16, tag="bv")
                    w3_n = io.tile([LANES, NINTH * PPC], BF16, tag="w3")
                    c12_n = io.tile([LANES, NINTH * PPC], BF16, tag="c12")
                    for c in range(NCHUNK):
                        nc.sync.dma_start(bv_n[:, bass.ts(c, CNB)],
                                          d_bv[o + 1][:, bass.ts(c, CNB)])
                    nc.sync.dma_start(w3_n[:], d_w3[o + 1][:])
                    nc.sync.dma_start(c12_n[:], d_c12[o + 1][:])
                else:
                    bv_n = w3_n = c12_n = None

                q_t = wk.tile([LANES, NINTH * NB], BF16, tag="q")
                t_t = wk.tile([LANES, NINTH * NB], BF16, tag="t")
                q4 = q_t[:].rearrange("l (n b p) -> l n b p", b=B, p=PPC)
                t4 = t_t[:].rearrange("l (n b p) -> l n b p", b=B, p=PPC)
                bv4 = bv_t[:].rearrange("l (n b p) -> l n b p", b=B, p=PPC)
                w3r = w3_t[:].rearrange("l (n p) -> l n p", p=PPC)
                c12r = c12_t[:].rearrange("l (n p) -> l n p", p=PPC)
                def bc(r, csl_):
                    return r[:, csl_].unsqueeze(2).broadcast_to([LANES, CH, B, PPC])

                # emit q-mults first, then adds (Pool-offloaded one leads so it
                # overlaps DVE's remaining work), then t-mults
                for c in range(NCHUNK):
                    csl = slice(c * CH, (c + 1) * CH)
                    nc.vector.tensor_tensor(q4[:, csl], bv4[:, csl],
                                            bc(w3r, csl), mul)
                order = sorted(range(NCHUNK),
                               key=lambda c: 0 if (o, c) in POOL_ADD else 1)
                for c in order:
                    csl = slice(c * CH, (c + 1) * CH)
                    eng = nc.gpsimd if (o, c) in POOL_ADD else nc.vector
                    eng.tensor_tensor(q4[:, csl], q4[:, csl], bc(c12r, csl), add)
                for c in range(NCHUNK):
                    cs = bass.ts(c, CNB)
                    nc.vector.tensor_tensor(t_t[:, cs], q_t[:, cs], ar[:, cs], mul)

                for n in range(NINTH):
                    last = (o == NOCT - 1) and (n == NINTH - 1)
                    nc.tensor.matmul(
                        psum[:], sred[:, bass.ts(o, 32)], t_t[:, bass.ts(n, NB)],
                        start=first_mm, stop=last, skip_group_check=True)
                    first_mm = False
                bv_t, w3_t, c12_t = bv_n, w3_n, c12_n

            # out = psum + bias (broadcast over b; GpSimd can't read PSUM -> DVE)
            psum3 = psum[:].rearrange("q (b p) -> q b p", p=PPC)
            out3 = out_sb[:].rearrange("q (b p) -> q b p", p=PPC)
            biasb = biast[:].unsqueeze(1).broadcast_to([32, B, PPC])
            nc.vector.tensor_tensor(out3, psum3, biasb, add)
            nc.sync.dma_start(d_out[:, :], out_sb[:])
    nc.finalize()
    return nc


def _prep(x, weight, mask):
    x = np.ascontiguousarray(np.asarray(x, dtype=np.float32))
    weight = np.ascontiguousarray(np.asarray(weight, dtype=np.float32))
    mask = np.asarray(mask, dtype=np.int64)
    bf = ml_dtypes.bfloat16

    # within-receptive-field index of LUT input 2
    m = mask.reshape(COUT, P, F, 2, 3)
    pr = (np.arange(P) // WOUT)[None, :, None]
    pc = (np.arange(P) % WOUT)[None, :, None]
    g = (m[..., 0] * KS + (m[..., 1] - pr[..., None])) * KS + (m[..., 2] - pc[..., None])
    sel2 = g[..., 1].astype(np.int64)                # (COUT,P,F)

    # im2col E[b,p,f]
    E = np.empty((B, P, F), dtype=np.float32)
    xv = x.reshape(B, CIN, H, W)
    for gg in range(F):
        cch, rem = divmod(gg, KS * KS)
        ddr, ddc = divmod(rem, KS)
        E[:, :, gg] = xv[:, cch, ddr:ddr + HOUT, ddc:ddc + WOUT].reshape(B, P)

    # gather of input-2 values: bvals[b,co,p,f] = E[b,p,sel2[co,p,f]]
    flat_idx = (np.arange(P)[None, :, None] * F + sel2).reshape(-1)
    bvals = E.reshape(B, P * F)[:, flat_idx].reshape(B, COUT, P, F)

    # weight-only preprocessing: butterfly + scatter + bias (offline-foldable)
    w4 = weight.reshape(COUT, P, F, 4)
    w0, w1, w2, w3_ = w4[..., 0], w4[..., 1], w4[..., 2], w4[..., 3]
    c0 = w0 + w1 + w2 + w3_
    c1 = -w0 - w1 + w2 + w3_
    c2 = -w0 + w1 - w2 + w3_
    c3 = w0 - w1 - w2 + w3_
    bias = c0.sum(-1)                                # (COUT,P)
    c12 = c1.copy()
    base = (np.arange(COUT * P) * F)[:, None]
    np.add.at(c12.reshape(-1), (base + sel2.reshape(COUT * P, F)).ravel(),
              c2.reshape(COUT * P, F).ravel())

    # one-hot reduction stationaries: S[lane, o*32 + (8o + lane//16)] = 1
    lane_co = (np.arange(LANES) // 16)
    sred = np.zeros((LANES, NOCT * 32), dtype=bf)
    for o in range(NOCT):
        sred[np.arange(LANES), o * 32 + 8 * o + lane_co] = 1.0

    in_maps = []
    for k in range(NCORE):
        ps_, ncnt = PSTART[k], PCNT[k]
        sl = slice(ps_, ps_ + ncnt)
        Ek = np.zeros((B, PPC, F), dtype=np.float32)
        Ek[:, :ncnt] = E[:, sl]
        bvk = np.zeros((B, COUT, PPC, F), dtype=np.float32)
        bvk[:, :, :ncnt] = bvals[:, :, sl]
        c3k = np.zeros((COUT, PPC, F), dtype=np.float32)
        c3k[:, :ncnt] = c3[:, sl]
        c12k = np.zeros((COUT, PPC, F), dtype=np.float32)
        c12k[:, :ncnt] = c12[:, sl]
        biask = np.zeros((COUT, PPC), dtype=np.float32)
        biask[:, :ncnt] = bias[:, sl]

        im = {"sred": sred, "biasv": biask}
        E_l = Ek.reshape(B, PPC, NINTH, 16)          # (b,p,n,j)
        im["ar"] = np.ascontiguousarray(
            np.broadcast_to(E_l.transpose(3, 2, 0, 1)[None], (8, 16, NINTH, B, PPC))
        ).reshape(LANES, NINTH * NB).astype(bf)
        for o in range(NOCT):
            cosl = slice(8 * o, 8 * o + 8)
            bvo = bvk[:, cosl].reshape(B, 8, PPC, NINTH, 16)
            im[f"bv_{o}"] = np.ascontiguousarray(
                bvo.transpose(1, 4, 3, 0, 2)).reshape(LANES, NINTH * NB).astype(bf)
            im[f"w3_{o}"] = np.ascontiguousarray(
                c3k[cosl].reshape(8, PPC, NINTH, 16).transpose(0, 3, 2, 1)
            ).reshape(LANES, NINTH * PPC).astype(bf)
            im[f"c12_{o}"] = np.ascontiguousarray(
                c12k[cosl].reshape(8, PPC, NINTH, 16).transpose(0, 3, 2, 1)
            ).reshape(LANES, NINTH * PPC).astype(bf)
        in_maps.append(im)
    return in_maps


def kernel(x, weight, mask):
    if "nc" not in _cache:
        _cache["nc"] = _build()
    nc = _cache["nc"]
    in_maps = _prep(x, weight, mask)
    res = run_bass_kernel_spmd(nc, in_maps, core_ids=list(range(NCORE)))
    out = np.empty((B, COUT, HOUT, WOUT), dtype=np.float32)
    ov = out.reshape(B, COUT, P)
    for k in range(NCORE):
        dev = res.results[k]["out"].reshape(32, B, PPC)          # (co, b, p)
        ov[:, :, PSTART[k]:PSTART[k] + PCNT[k]] = dev.transpose(1, 0, 2)[:, :, :PCNT[k]]
    return out


if __name__ == "__main__":
    print("kernel module ok")


# revision 9
# speedup vs baseline: 2.4838x; 1.1240x over previous
"""TRN2 Bass kernel for nn_Conv2d_62826781606523 (LUT-conv, K_LUT=2).

Math per table t=(co,p,f), weights w[t,0:4], inputs a=E[b,p,f], bb=E[b,p,sel2]:
    out_t = c0 + c1*a + c2*bb + c3*a*bb         (butterfly of w)
    out[b,co,p] = sum_f out_t
Host-side weight-only preprocessing (offline-foldable):
    bias[co,p] = sum_f c0;   c12 = c1 + scatter_add(c2 over sel2)
so   out[b,co,p] = bias + sum_f (c3*bb + c12) * a.

Device split (p-sharded across 8 cores, PPC=113 positions/core):
  DMA   : bv streamed as fp8_e4m3 (2.1 MB/core), weights + ar as bf16
  Scalar: expands fp8 bv -> bf16 (keeps DVE in 2x mode)
  DVE   : q = c3*bv + c12 (b-bcast weights), t = q*ar    [bf16 2x mode]
  PE    : segmented f-reduction of t via one-hot stationaries, accumulating
          4 octets x 9 ninths into one PSUM tile (32co x 452(b,p))
  Output: psum + bias (DVE) -> SBUF -> DMA.
Layout: partitions = (co_local8, f16) per co-octet; f = 144 -> 9 "ninths".
Host does layout only for x-dependent data (im2col, static-index gather,
fp8/bf16 pack, padding); no x-dependent arithmetic on host.
"""
import numpy as np
import ml_dtypes

import concourse.bass as bass
import concourse.bacc as bacc
import concourse.mybir as mybir
from concourse.bass_types import AP
from concourse.tile import TileContext
from concourse.bass_utils import run_bass_kernel_spmd

# problem constants (hardcoded per task contract)
B, CIN, COUT, KS, H, W = 4, 16, 32, 3, 32, 32
HOUT = WOUT = 30
P = HOUT * WOUT          # 900
F = CIN * KS * KS        # 144
T = COUT * P * F
NCORE = 8
PPC = 113                # padded p positions per core
NB = PPC * B             # 452 columns (b-major: col = b*PPC + p), <=512 moving
NOCT = 4                 # co octets of 8
NINTH = 9                # f = 9 * 16
LANES = 128              # (co_local 8) x (f16)
BF16 = mybir.dt.bfloat16
FP8 = mybir.dt.float8e4
F32 = mybir.dt.float32

PCNT = [113, 113, 113, 113, 112, 112, 112, 112]
PSTART = np.concatenate([[0], np.cumsum(PCNT)[:-1]]).astype(int)

CH = 3                   # ninths per DMA / fp8-expansion chunk
NCHUNK = NINTH // CH
CNB = CH * NB

_cache = {}


def _build():
    nc = bacc.Bacc()
    d_w3 = [nc.dram_tensor(f"w3_{o}", [LANES, NINTH * PPC], BF16, kind="ExternalInput")
            for o in range(NOCT)]
    d_c12 = [nc.dram_tensor(f"c12_{o}", [LANES, NINTH * PPC], BF16,
                            kind="ExternalInput")
             for o in range(NOCT)]
    d_bv = [nc.dram_tensor(f"bv_{o}", [LANES, NINTH * NB], FP8, kind="ExternalInput")
            for o in range(NOCT)]
    d_ar = nc.dram_tensor("ar", [LANES, NINTH * NB], BF16, kind="ExternalInput")
    d_bias = nc.dram_tensor("biasv", [32, PPC], F32, kind="ExternalInput")
    d_s = nc.dram_tensor("sred", [LANES, NOCT * 32], BF16, kind="ExternalInput")
    d_out = nc.dram_tensor("out", [32, NB], F32, kind="ExternalOutput")

    mul = mybir.AluOpType.mult
    add = mybir.AluOpType.add

    with TileContext(nc) as tc:
        with (
            tc.tile_pool(name="cst", bufs=1) as cst,
            tc.tile_pool(name="io", bufs=2) as io,
            tc.tile_pool(name="wk", bufs=2) as wk,
            tc.psum_pool(name="ps", bufs=1) as ps,
        ):
            # resident tiles
            ar = cst.tile([LANES, NINTH * NB], BF16, name="art")
            biast = cst.tile([32, PPC], F32, name="biastt")
            sred = cst.tile([LANES, NOCT * 32], BF16, name="sredt")
            psum = ps.tile([32, NB], F32, name="psumt")
            out_sb = cst.tile([32, NB], F32, name="outsb")

            # first octet + shared streams
            bv80 = io.tile([LANES, NINTH * NB], FP8, tag="bv8")
            w30 = io.tile([LANES, NINTH * PPC], BF16, tag="w3")
            c120 = io.tile([LANES, NINTH * PPC], BF16, tag="c12")
            for c in range(NCHUNK):
                nc.sync.dma_start(bv80[:, bass.ts(c, CNB)], d_bv[0][:, bass.ts(c, CNB)])
                nc.sync.dma_start(ar[:, bass.ts(c, CNB)], d_ar[:, bass.ts(c, CNB)])
            nc.sync.dma_start(w30[:], d_w3[0][:])
            nc.sync.dma_start(c120[:], d_c12[0][:])
            nc.sync.dma_start(sred[:], d_s[:])
            nc.sync.dma_start(biast[:], d_bias[:])

            first_mm = True
            bv8_t, w3_t, c12_t = bv80, w30, c120
            for o in range(NOCT):
                if o + 1 < NOCT:
                    bv8_n = io.tile([LANES, NINTH * NB], FP8, tag="bv8")
                    w3_n = io.tile([LANES, NINTH * PPC], BF16, tag="w3")
                    c12_n = io.tile([LANES, NINTH * PPC], BF16, tag="c12")
                    for c in range(NCHUNK):
                        nc.sync.dma_start(bv8_n[:, bass.ts(c, CNB)],
                                          d_bv[o + 1][:, bass.ts(c, CNB)])
                    nc.sync.dma_start(w3_n[:], d_w3[o + 1][:])
                    nc.sync.dma_start(c12_n[:], d_c12[o + 1][:])
                else:
                    bv8_n = w3_n = c12_n = None

                # Scalar engine: expand fp8 -> bf16 per chunk
                bvx = wk.tile([LANES, NINTH * NB], BF16, tag="bvx")
                for c in range(NCHUNK):
                    cs = bass.ts(c, CNB)
                    nc.scalar.copy(bvx[:, cs], bv8_t[:, cs])

                q_t = wk.tile([LANES, NINTH * NB], BF16, tag="q")
                t_t = wk.tile([LANES, NINTH * NB], BF16, tag="t")
                q4 = q_t[:].rearrange("l (n b p) -> l n b p", b=B, p=PPC)
                bv4 = bvx[:].rearrange("l (n b p) -> l n b p", b=B, p=PPC)
                w3b = w3_t[:].rearrange("l (n p) -> l n p", p=PPC) \
                    .unsqueeze(2).broadcast_to([LANES, NINTH, B, PPC])
                c12b = c12_t[:].rearrange("l (n p) -> l n p", p=PPC) \
                    .unsqueeze(2).broadcast_to([LANES, NINTH, B, PPC])
                nc.vector.tensor_tensor(q4, bv4, w3b, mul)
                nc.vector.tensor_tensor(q4, q4, c12b, add)
                nc.vector.tensor_tensor(t_t[:], q_t[:], ar[:], mul)

                for n in range(NINTH):
                    last = (o == NOCT - 1) and (n == NINTH - 1)
                    nc.tensor.matmul(
                        psum[:], sred[:, bass.ts(o, 32)], t_t[:, bass.ts(n, NB)],
                        start=first_mm, stop=last, skip_group_check=True)
                    first_mm = False
                bv8_t, w3_t, c12_t = bv8_n, w3_n, c12_n

            # out = psum + bias (broadcast over b) on DVE, then DMA
            psum3 = psum[:].rearrange("q (b p) -> q b p", p=PPC)
            out3 = out_sb[:].rearrange("q (b p) -> q b p", p=PPC)
            biasb = biast[:].unsqueeze(1).broadcast_to([32, B, PPC])
            nc.vector.tensor_tensor(out3, psum3, biasb, add)
            nc.sync.dma_start(d_out[:, :], out_sb[:])
    nc.finalize()
    return nc


def _prep(x, weight, mask):
    x = np.ascontiguousarray(np.asarray(x, dtype=np.float32))
    weight = np.ascontiguousarray(np.asarray(weight, dtype=np.float32))
    mask = np.asarray(mask, dtype=np.int64)
    bf = ml_dtypes.bfloat16
    f8 = ml_dtypes.float8_e4m3fn

    # within-receptive-field index of LUT input 2
    m = mask.reshape(COUT, P, F, 2, 3)
    pr = (np.arange(P) // WOUT)[None, :, None]
    pc = (np.arange(P) % WOUT)[None, :, None]
    g = (m[..., 0] * KS + (m[..., 1] - pr[..., None])) * KS + (m[..., 2] - pc[..., None])
    sel2 = g[..., 1].astype(np.int64)                # (COUT,P,F)

    # im2col E[b,p,f]
    E = np.empty((B, P, F), dtype=np.float32)
    xv = x.reshape(B, CIN, H, W)
    for gg in range(F):
        cch, rem = divmod(gg, KS * KS)
        ddr, ddc = divmod(rem, KS)
        E[:, :, gg] = xv[:, cch, ddr:ddr + HOUT, ddc:ddc + WOUT].reshape(B, P)

    # gather of input-2 values: bvals[b,co,p,f] = E[b,p,sel2[co,p,f]]
    flat_idx = (np.arange(P)[None, :, None] * F + sel2).reshape(-1)
    bvals = E.reshape(B, P * F)[:, flat_idx].reshape(B, COUT, P, F)

    # weight-only preprocessing: butterfly + scatter + bias (offline-foldable)
    w4 = weight.reshape(COUT, P, F, 4)
    w0, w1, w2, w3_ = w4[..., 0], w4[..., 1], w4[..., 2], w4[..., 3]
    c0 = w0 + w1 + w2 + w3_
    c1 = -w0 - w1 + w2 + w3_
    c2 = -w0 + w1 - w2 + w3_
    c3 = w0 - w1 - w2 + w3_
    bias = c0.sum(-1)                                # (COUT,P)
    c12 = c1.copy()
    base = (np.arange(COUT * P) * F)[:, None]
    np.add.at(c12.reshape(-1), (base + sel2.reshape(COUT * P, F)).ravel(),
              c2.reshape(COUT * P, F).ravel())

    # one-hot reduction stationaries: S[lane, o*32 + (8o + lane//16)] = 1
    lane_co = (np.arange(LANES) // 16)
    sred = np.zeros((LANES, NOCT * 32), dtype=bf)
    for o in range(NOCT):
        sred[np.arange(LANES), o * 32 + 8 * o + lane_co] = 1.0

    in_maps = []
    for k in range(NCORE):
        ps_, ncnt = PSTART[k], PCNT[k]
        sl = slice(ps_, ps_ + ncnt)
        Ek = np.zeros((B, PPC, F), dtype=np.float32)
        Ek[:, :ncnt] = E[:, sl]
        bvk = np.zeros((B, COUT, PPC, F), dtype=np.float32)
        bvk[:, :, :ncnt] = bvals[:, :, sl]
        c3k = np.zeros((COUT, PPC, F), dtype=np.float32)
        c3k[:, :ncnt] = c3[:, sl]
        c12k = np.zeros((COUT, PPC, F), dtype=np.float32)
        c12k[:, :ncnt] = c12[:, sl]
        biask = np.zeros((COUT, PPC), dtype=np.float32)
        biask[:, :ncnt] = bias[:, sl]

        im = {"sred": sred, "biasv": biask}
        E_l = Ek.reshape(B, PPC, NINTH, 16)          # (b,p,n,j)
        im["ar"] = np.ascontiguousarray(
            np.broadcast_to(E_l.transpose(3, 2, 0, 1)[None], (8, 16, NINTH, B, PPC))
        ).reshape(LANES, NINTH * NB).astype(bf)
        for o in range(NOCT):
            cosl = slice(8 * o, 8 * o + 8)
            bvo = bvk[:, cosl].reshape(B, 8, PPC, NINTH, 16)
            im[f"bv_{o}"] = np.ascontiguousarray(
                bvo.transpose(1, 4, 3, 0, 2)).reshape(LANES, NINTH * NB).astype(f8)
            im[f"w3_{o}"] = np.ascontiguousarray(
                c3k[cosl].reshape(8, PPC, NINTH, 16).transpose(0, 3, 2, 1)
            ).reshape(LANES, NINTH * PPC).astype(bf)
            im[f"c12_{o}"] = np.ascontiguousarray(
                c12k[cosl].reshape(8, PPC, NINTH, 16).transpose(0, 3, 2, 1)
            ).reshape(LANES, NINTH * PPC).astype(bf)
        in_maps.append(im)
    return in_maps


def kernel(x, weight, mask):
    if "nc" not in _cache:
        _cache["nc"] = _build()
    nc = _cache["nc"]
    in_maps = _prep(x, weight, mask)
    res = run_bass_kernel_spmd(nc, in_maps, core_ids=list(range(NCORE)))
    out = np.empty((B, COUT, HOUT, WOUT), dtype=np.float32)
    ov = out.reshape(B, COUT, P)
    for k in range(NCORE):
        dev = res.results[k]["out"].reshape(32, B, PPC)          # (co, b, p)
        ov[:, :, PSTART[k]:PSTART[k] + PCNT[k]] = dev.transpose(1, 0, 2)[:, :, :PCNT[k]]
    return out


if __name__ == "__main__":
    print("kernel module ok")
